# revision 1
# baseline (speedup 1.0000x reference)
"""Trainium2 Bass kernel for LoopRelationalGraphConvolution.

Math (matches the jax reference):
    out[n] = relu( SCALE * sum_s  W[rel[n,s]] @ emb[neighbors[n,s]] )
    SCALE  = 1000 / (R1 * S)      (folds the mean over S and the /R1 * 1000)

Design (8 NeuronCores, data-parallel over the 8192-node batch):
  Each core owns 1024 nodes, split into 9 node-tiles ([114]*8 + [112] nodes)
  chosen by a host-side balancer so that every (tile, relation) bucket has
  <=128 edges.  Per tile the device kernel:
    1. dma_gather(transpose=True): fetches the tile's 33*128 edge-slot
       embeddings (bf16, compacted per-core int16 ids) with the embedding dim
       landing on SBUF partitions:  ET[p, c, i] = emb[idx_i][c*128+p].
    2. stage-1 matmuls: per relation-chunk r (128 slots), in 2 K-chunks over D:
       Y[slot, o] += ET[:, c, slots]^T @ W_r[c]       (PSUM, f32)
    3. stage-2 matmul: 0/1 selection matrix reduces edge slots into node rows:
       out_psum[node, o] += SEL_r^T @ Y_bf16          (accumulated over all r)
    4. relu on PSUM->SBUF evacuation, DMA node rows to DRAM.
  The device program is fully static and identical across cores (SPMD); all
  data-dependence lives in the index / selection arrays.  Host post-step
  inverse-permutes rows back to the original node order.
"""

import numpy as np
import ml_dtypes

bf16 = ml_dtypes.bfloat16
fp8 = ml_dtypes.float8_e4m3

# Problem constants (hardcoded per contract).
V = 100000
D = 256
R1 = 33          # relations incl. self-loop
N = 8192
S = 32
NCORES = 8
NPC = N // NCORES          # 1024 nodes per core
NTILES = 9                 # node-tiles per core
CAPS = [114] * 8 + [112]   # nodes per tile (uniform across cores)
ROW_BASE = np.concatenate([[0], np.cumsum(CAPS)]).tolist()
P = 128
NSLOT = R1 * P             # 4224 edge slots per tile
GSPLIT = [0, 2, 6, 12, 19, 26, R1]   # gather segment chunk boundaries
GSEG = [(a * P, b * P) for a, b in zip(GSPLIT, GSPLIT[1:])]
IDXW = NSLOT // 16         # 264 int16 idx columns (16-partition wrap)
UMAX = 32768               # compacted per-core embedding rows (int16 limit)
SCALE = 1000.0 / (R1 * S)

# Software-pipeline skew between stage-1 and stage-2 of consecutive chunks,
# so the PE never stalls on the PSUM->SBUF copy of the current chunk.
SKEW = 6
PF = 3      # tile prefetch depth


# ---------------------------------------------------------------------------
# Host-side preparation
# ---------------------------------------------------------------------------

def _balance_tiles(hist):
    """Assign NPC nodes to NTILES tiles (exactly CAPS[t] nodes each),
    minimizing the max per-(tile, relation) edge count. hist: [NPC, R1].
    Greedy: hardest nodes first, place on the tile minimizing the resulting
    peak bucket."""
    order = np.argsort(-hist.max(axis=1), kind="stable")
    loads = np.zeros((NTILES, R1), dtype=np.int64)
    counts = np.zeros(NTILES, dtype=np.int64)
    tiles = [[] for _ in range(NTILES)]
    for n in order:
        h = hist[n]
        best_t, best_key = -1, None
        for t in range(NTILES):
            if counts[t] >= CAPS[t]:
                continue
            new = loads[t] + h
            key = (int(new.max()), int(loads[t].max()), int(new.sum()))
            if best_key is None or key < best_key:
                best_key, best_t = key, t
        tiles[best_t].append(int(n))
        loads[best_t] += h
        counts[best_t] += 1
    return tiles, loads


def prep(emb_table, weights, neighbors, relations):
    """Build per-core device arrays. Returns (in_maps, perms)."""
    emb_bf = np.asarray(emb_table).astype(bf16)
    w = np.asarray(weights, dtype=np.float32) * SCALE         # [R1, D_out, D_in]
    # W_sb[p, (r*2 + c)*D + o] = w[r, o, c*128+p]
    w_rdo = np.ascontiguousarray(w.transpose(0, 2, 1))        # [r, d, o]
    W_sb = np.ascontiguousarray(
        w_rdo.reshape(R1, 2, 128, D).transpose(2, 0, 1, 3)    # [p, r, c, o]
    ).reshape(128, R1 * 2 * D).astype(bf16)

    neighbors = np.asarray(neighbors).astype(np.int64)
    relations = np.asarray(relations).astype(np.int64)

    in_maps, perms = [], []
    for c in range(NCORES):
        nb = neighbors[c * NPC:(c + 1) * NPC]                 # [NPC, S]
        rel = relations[c * NPC:(c + 1) * NPC]
        uniq, inv = np.unique(nb.ravel(), return_inverse=True)
        inv = inv.reshape(nb.shape).astype(np.int64)
        U = len(uniq)
        assert U <= UMAX, U
        emb_c = np.zeros((UMAX, D), dtype=bf16)
        emb_c[:U] = emb_bf[uniq]

        hist = np.zeros((NPC, R1), dtype=np.int64)
        np.add.at(hist, (np.repeat(np.arange(NPC), S), rel.ravel()), 1)
        tiles, loads = _balance_tiles(hist)
        assert loads.max() <= P, f"balance failed: max bucket {loads.max()}"

        idx_all = np.zeros((NTILES, 128, IDXW), dtype=np.int16)
        sel_all = np.zeros((NTILES, 128, NSLOT), dtype=fp8)
        perm = []
        for t, nodes in enumerate(tiles):
            nodes = np.array(nodes, dtype=np.int64)
            ncnt = len(nodes)
            assert ncnt == CAPS[t]
            perm.extend((c * NPC + nodes).tolist())
            # edges of this tile
            er = rel[nodes].ravel()                            # relation per edge
            ei = inv[nodes].ravel()                            # compact nbr id
            ej = np.repeat(np.arange(ncnt), S)                 # local node idx
            order = np.argsort(er, kind="stable")
            er_s, ei_s, ej_s = er[order], ei[order], ej[order]
            # position within relation group
            start = np.searchsorted(er_s, np.arange(R1))
            pos = np.arange(ncnt * S) - start[er_s]
            slot = er_s * P + pos                              # [ncnt*S]
            slots_idx = np.zeros(NSLOT, dtype=np.int16)
            slots_idx[slot] = ei_s
            sel = np.zeros((NSLOT, 128), dtype=fp8)
            sel[slot, ej_s] = 1.0
            # idx wrap per gather segment: idx i at partition i%16, col i//16
            wrapped = np.concatenate(
                [slots_idx[a:b].reshape((b - a) // 16, 16).T
                 for a, b in GSEG], axis=1)                    # [16, IDXW]
            idx_all[t] = np.tile(wrapped, (8, 1))
            # device SEL layout: [part p = slot-in-chunk, free = r*128 + node]
            sel_all[t] = np.ascontiguousarray(
                sel.reshape(R1, P, 128).transpose(1, 0, 2).reshape(P, NSLOT))
        in_maps.append({
            "emb": emb_c,
            "wsb": W_sb,
            "idx": np.ascontiguousarray(idx_all.reshape(NTILES * 128, IDXW)),
            "sel": np.ascontiguousarray(sel_all.reshape(NTILES * 128, NSLOT)),
        })
        perms.append(np.array(perm, dtype=np.int64))

    return in_maps, perms


# ---------------------------------------------------------------------------
# Numpy emulation (bf16-faithful) for validation
# ---------------------------------------------------------------------------

def emulate_core(in_map):
    emb = in_map["emb"]                                        # [UMAX, D] bf16
    wsb = in_map["wsb"].reshape(128, R1, 2, D)                 # [p, r, c, o]
    idx = in_map["idx"].reshape(NTILES, 128, IDXW)
    sel = in_map["sel"].reshape(NTILES, 128, NSLOT)
    out = np.zeros((NPC, D), dtype=np.float32)
    for t in range(NTILES):
        parts, col = [], 0
        for a, b in GSEG:
            w = (b - a) // 16
            parts.append(idx[t, :16, col:col + w].T.reshape(b - a))
            col += w
        slots_idx = np.concatenate(parts)                      # unwrap
        X = emb[slots_idx]                                     # [NSLOT, D] bf16
        out_acc = np.zeros((128, D), dtype=np.float32)
        for r in range(R1):
            Xr = X[r * P:(r + 1) * P].astype(np.float32)       # [128, D]
            Y = (Xr[:, :128] @ wsb[:, r, 0, :].astype(np.float32)
                 + Xr[:, 128:] @ wsb[:, r, 1, :].astype(np.float32))
            Yb = Y.astype(bf16).astype(np.float32)             # PSUM->SBUF bf16
            selr = sel[t][:, r * 128:(r + 1) * 128].astype(np.float32)
            out_acc += selr.T @ Yb
        base, ncnt = ROW_BASE[t], CAPS[t]
        out[base:base + ncnt] = np.maximum(out_acc[:ncnt], 0.0)
    return out


def emulate(emb_table, weights, neighbors, relations):
    in_maps, perms = prep(emb_table, weights, neighbors, relations)
    full = np.zeros((N, D), dtype=np.float32)
    for c in range(NCORES):
        full[perms[c]] = emulate_core(in_maps[c])
    return full


# ---------------------------------------------------------------------------
# Bass program
# ---------------------------------------------------------------------------

def build_program():
    import concourse.bacc as bacc
    import concourse.tile as tile
    import concourse.mybir as mybir

    nc = bacc.Bacc(
        "TRN2", target_bir_lowering=False, debug=False,
        num_devices=NCORES,
    )
    BF = mybir.dt.bfloat16
    F32 = mybir.dt.float32
    I16 = mybir.dt.int16
    F8 = mybir.dt.float8e4

    emb = nc.dram_tensor("emb", [UMAX, D], BF, kind="ExternalInput").ap()
    wsb = nc.dram_tensor("wsb", [128, R1 * 2 * D], BF, kind="ExternalInput").ap()
    idx = nc.dram_tensor("idx", [NTILES * 128, IDXW], I16, kind="ExternalInput").ap()
    sel = nc.dram_tensor("sel", [NTILES * 128, NSLOT], F8,
                         kind="ExternalInput").ap()
    out = nc.dram_tensor("out", [NPC, D], F32, kind="ExternalOutput").ap()

    Relu = mybir.ActivationFunctionType.Relu

    with tile.TileContext(nc) as tc:
        with (
            tc.tile_pool(name="wpool", bufs=1) as wpool,
            tc.tile_pool(name="etpool", bufs=PF + 1) as etpool,
            tc.tile_pool(name="selpool", bufs=PF + 1) as selpool,
            tc.tile_pool(name="idxpool", bufs=PF + 1) as idxpool,
            tc.tile_pool(name="ypool", bufs=2 * (SKEW + 1)) as ypool,
            tc.tile_pool(name="opool", bufs=2) as opool,
            tc.tile_pool(name="psy", bufs=SKEW + 1, space="PSUM") as psy,
            tc.tile_pool(name="pso", bufs=1, space="PSUM") as pso,
        ):
            wt = wpool.tile([128, R1 * 2 * D], BF)

            def load_w(r0, r1):
                for r in range(r0, r1):
                    a, b = r * 2 * D, (r + 1) * 2 * D
                    nc.sync.dma_start(out=wt[:, a:b], in_=wsb[:, a:b])

            ets, sels = {}, {}

            def pre_gather(t):
                idx_t = idxpool.tile([128, IDXW], I16, name="idx_t")
                nc.sync.dma_start(
                    out=idx_t[:], in_=idx[t * 128:(t + 1) * 128, :])
                segs, col = [], 0
                for gi, (a, b) in enumerate(GSEG):
                    n = b - a
                    eth = etpool.tile([128, 2, n], BF, name=f"et{gi}")
                    nc.gpsimd.dma_gather(
                        out_ap=eth[:],
                        in_ap=emb,
                        idxs_ap=idx_t[:, col:col + n // 16],
                        num_idxs=n,
                        num_idxs_reg=n,
                        elem_size=D,
                        transpose=True,
                        single_packet=False,
                    )
                    col += n // 16
                    segs.append(eth)
                ets[t] = segs

            def pre_sel(t):
                sel_t = selpool.tile([128, NSLOT], F8, name="sel_t")
                nc.sync.dma_start(
                    out=sel_t[:], in_=sel[t * 128:(t + 1) * 128, :])
                sels[t] = sel_t

            def prefetch(t):
                if t >= NTILES:
                    return
                pre_gather(t)
                pre_sel(t)

            # startup orchestration: gather(0) first, early W chunks, sel(0),
            # then stream the rest so the PE can start by ~9us.
            pre_gather(0)
            load_w(0, 11)
            pre_sel(0)
            pre_gather(1)
            load_w(11, R1)
            pre_sel(1)
            prefetch(2)

            for t in range(NTILES):
                segs, sel_t = ets.pop(t), sels.pop(t)
                outp = pso.tile([128, D], F32)
                ys = [None] * R1
                for r in range(R1 + SKEW):
                    if r < R1:
                        yp = psy.tile([128, D], F32, name="yp")
                        gi = next(i for i, (a, b) in enumerate(GSEG)
                                  if a <= r * P < b)
                        eth, off = segs[gi], r * P - GSEG[gi][0]
                        for c in range(2):
                            nc.tensor.matmul(
                                out=yp[:],
                                lhsT=eth[:, c, off:off + P],
                                rhs=wt[:, (r * 2 + c) * D:(r * 2 + c + 1) * D],
                                start=(c == 0), stop=(c == 1),
                            )
                        ysb = ypool.tile([128, D], BF, name="ysb")
                        if r % 2 == 0:
                            nc.vector.tensor_copy(out=ysb[:], in_=yp[:])
                        else:
                            nc.scalar.copy(out=ysb[:], in_=yp[:])
                        ys[r] = ysb
                    if r >= SKEW:
                        q = r - SKEW
                        nc.tensor.matmul(
                            out=outp[:],
                            lhsT=sel_t[:, q * P:(q + 1) * P],
                            rhs=ys[q][:],
                            start=(q == 0), stop=(q == R1 - 1),
                        )
                prefetch(t + PF)
                osb = opool.tile([128, D], F32)
                nc.scalar.activation(out=osb[:], in_=outp[:], func=Relu)
                base, ncnt = ROW_BASE[t], CAPS[t]
                nc.sync.dma_start(
                    out=out[base:base + ncnt, :], in_=osb[:ncnt, :])

    nc.compile()
    return nc


_NC_CACHE = []


def _get_program():
    if not _NC_CACHE:
        _NC_CACHE.append(build_program())
    return _NC_CACHE[0]


# ---------------------------------------------------------------------------
# Entry point
# ---------------------------------------------------------------------------

def kernel(emb_table, weights, neighbors, relations):
    from concourse import bass_utils

    in_maps, perms = prep(emb_table, weights, neighbors, relations)
    nc = _get_program()
    res = bass_utils.run_bass_kernel_spmd(
        nc, in_maps, core_ids=list(range(NCORES)),
    )
    full = np.zeros((N, D), dtype=np.float32)
    for c in range(NCORES):
        full[perms[c]] = res.results[c]["out"]
    return full



# revision 44
# speedup vs baseline: 1.1197x; 1.1197x over previous
"""Trainium2 Bass kernel for LoopRelationalGraphConvolution.

Math (matches the jax reference):
    out[n] = relu( SCALE * sum_s  W[rel[n,s]] @ emb[neighbors[n,s]] )
    SCALE  = 1000 / (R1 * S)      (folds the mean over S and the /R1 * 1000)

Design (8 NeuronCores, data-parallel over the 8192-node batch), aggregate-first:
  Each core owns 1024 nodes in 8 node-tiles of 128.  Per tile, relation r's
  edges occupy gather group r (slots 128r..128r+127).  Buckets with more than
  128 edges are reduced by pairing two edges of the SAME (node, relation):
  the pair becomes one 512B table row [fp8_hi(u) | fp8_hi(v)]; normal rows are
  [fp8_hi(emb) | fp8_lo(emb - hi)].  Per tile the device kernel:
    1. dma_gather (transpose=False): slot i lands on partition i%128, group
       i//128; each slot's 512B row is contiguous in the free dim.
    2. aggregation matmuls (fp8 DoubleRow): per (r, d-chunk c) one DR matmul
       contracts k-tiles (bytes 0:256, bytes 256:512) against a stride-0
       broadcast of the 0/1 sel matrix:
         aggT[d, node] = sum_slots (plane0 + plane1)[d] * sel[slot, node]
       exact f32 PSUM accumulation; 128-col DR output = 26.7ns each.
    3. evac aggT (two relations share one PSUM bank) -> SBUF bf16.
    4. stage-B matmuls (bf16): out[node, o] += aggT[:, c, :]^T @ W[r, c]
       accumulated over all (r, c) in PSUM.
    5. relu on PSUM->SBUF evacuation (bf16), DMA node rows to DRAM.
  The device program is static and identical across cores (SPMD); all data
  dependence lives in the index / selection arrays.  Host post-step inverse-
  permutes rows back to the original node order.
"""

import numpy as np
import ml_dtypes

bf16 = ml_dtypes.bfloat16
fp8 = ml_dtypes.float8_e4m3
fp8e5 = ml_dtypes.float8_e5m2

# Problem constants (hardcoded per contract).
V = 100000
D = 256
R1 = 33          # relations incl. self-loop
N = 8192
S = 32
NCORES = 8
NPC = N // NCORES          # 1024 nodes per core
NTILES = 8                 # node-tiles per core
TN = 128                   # nodes per tile
P = 128
NSLOT = R1 * P             # 4224 edge slots per tile
# gather segments (group boundaries): tile 0 split fine for fast startup,
# last tile ends with a small segment to shorten the data-dependent tail
GSPLIT0 = [0, 2, 6, 14, 24, R1]
GSPLIT = [0, 16, R1]
GSPLITL = [0, 16, 28, R1]


def tile_gsplit(t):
    if t == 0:
        return GSPLIT0
    if t == NTILES - 1:
        return GSPLITL
    return GSPLIT
IDXW = NSLOT // 16         # 264 int16 idx columns (16-partition wrap)
UMAX = 32768               # compacted per-core embedding rows (int16 limit)
SCALE = 1000.0 / (R1 * S)

# Software-pipeline skew (in relation PAIRS) between aggregation and stage-B,
# so the PE never stalls on the PSUM->SBUF copy.
SKEW = 4                   # agg -> evac PSUM depth (PSUM-bank limited)
LAG = 4                    # agg -> stage-B lag in pairs (SBUF-buffered)
LAG_END = 4                # shrunken lag near the stream end (short tail)
NPAIR = (R1 + 1) // 2      # 17 relation pairs per tile (last is a single)
CONV_PAIRS = frozenset((9,))  # fp8 DoubleRow stage-B pairs
CONV_RELS = tuple(sorted(r for k in CONV_PAIRS for r in (2 * k, 2 * k + 1)))
BF_RELS = tuple(r for r in range(R1) if r not in CONV_RELS)
CIDX = {r: i for i, r in enumerate(CONV_RELS)}
BIDX = {r: i for i, r in enumerate(BF_RELS)}
R8 = len(CONV_RELS)        # 14 relations with fp8 stage-B (W as fp8 hi+lo)
RB = len(BF_RELS)          # 19 relations with bf16 stage-B
PF = 3      # tile prefetch depth
WARMUP = 54  # PE warmup matmuls (p-state ramp + cover first gather latency)


# ---------------------------------------------------------------------------
# Host-side preparation
# ---------------------------------------------------------------------------

def _balance_tiles(hist):
    """Assign NPC nodes to NTILES tiles (TN nodes each), minimizing the max
    per-(tile, relation) SLOT need, where a (node, rel) multiplicity k can be
    compressed to ceil(k/2) slots by same-bucket pairing.  hist: [NPC, R1].
    Returns tiles (lists of node ids)."""
    order = np.argsort(-hist.max(axis=1), kind="stable")
    loads = np.zeros((NTILES, R1), dtype=np.int64)     # raw edge counts
    pav = np.zeros((NTILES, R1), dtype=np.int64)       # pairs available
    counts = np.zeros(NTILES, dtype=np.int64)
    tiles = [[] for _ in range(NTILES)]
    for n in order:
        h = hist[n]
        hp = h // 2
        best_t, best_key = -1, None
        for t in range(NTILES):
            if counts[t] >= TN:
                continue
            nl = loads[t] + h
            np_ = pav[t] + hp
            need = nl - np.minimum(np_, np.maximum(nl - P, 0))
            key = (int(need.max()), int(nl.max()), int(nl.sum()))
            if best_key is None or key < best_key:
                best_key, best_t = key, t
        tiles[best_t].append(int(n))
        loads[best_t] += h
        pav[best_t] += hp
        counts[best_t] += 1
    need = loads - np.minimum(pav, np.maximum(loads - P, 0))
    assert need.max() <= P, f"balance failed: max need {need.max()}"
    return tiles


def _wrap_idx(slots_idx, gsplit):
    """Per-segment 16-partition wrap: segment [a, b) of the slot-index vector
    becomes idx[i%16, a//16 + i//16]."""
    parts = []
    for a, b in zip(gsplit, gsplit[1:]):
        a, b = a * P, b * P
        parts.append(slots_idx[a:b].reshape((b - a) // 16, 16).T)
    return np.concatenate(parts, axis=1)        # [16, IDXW]


def prep(emb_table, weights, neighbors, relations):
    """Build per-core device arrays. Returns (in_maps, perms)."""
    emb_f = np.asarray(emb_table, dtype=np.float32)
    w = np.asarray(weights, dtype=np.float32) * SCALE         # [R1, D_out, D_in]
    # W_sb[p, (r*2 + c)*D + o] = w[r, o, c*128+p]
    w_rdo = np.ascontiguousarray(w.transpose(0, 2, 1))        # [r, d, o]
    W_pc = np.ascontiguousarray(
        w_rdo.reshape(R1, 2, 128, D).transpose(2, 0, 1, 3))   # [p, r, c, o]
    W_sb = np.ascontiguousarray(
        W_pc[:, list(BF_RELS)]).reshape(128, RB * 2 * D).astype(bf16)
    w8 = np.ascontiguousarray(W_pc[:, list(CONV_RELS)]).astype(np.float32)
    W8h = w8.astype(fp8)
    W8l = (w8 - W8h.astype(np.float32)).astype(fp8e5)
    W8h = np.ascontiguousarray(W8h.reshape(128, R8 * 2 * D))
    W8l = np.ascontiguousarray(W8l.reshape(128, R8 * 2 * D))

    neighbors = np.asarray(neighbors).astype(np.int64)
    relations = np.asarray(relations).astype(np.int64)

    in_maps, perms = [], []
    for c in range(NCORES):
        nb = neighbors[c * NPC:(c + 1) * NPC]                 # [NPC, S]
        rel = relations[c * NPC:(c + 1) * NPC]
        uniq, inv = np.unique(nb.ravel(), return_inverse=True)
        inv = inv.reshape(nb.shape).astype(np.int64)
        U = len(uniq)
        eh = emb_f[uniq].astype(fp8)                          # [U, D] hi
        el = (emb_f[uniq] - eh.astype(np.float32)).astype(fp8)  # lo
        rows = [np.concatenate([eh, el], axis=1)]             # [U, 2D]
        nrows = U

        hist = np.zeros((NPC, R1), dtype=np.int64)
        np.add.at(hist, (np.repeat(np.arange(NPC), S), rel.ravel()), 1)
        tiles = _balance_tiles(hist)

        idx_all = np.zeros((NTILES, 128, IDXW), dtype=np.int16)
        sel_all = np.zeros((NTILES, 128, NSLOT), dtype=fp8)
        pair_rows = []
        perm = []
        for t, nodes in enumerate(tiles):
            nodes = np.array(nodes, dtype=np.int64)
            assert len(nodes) == TN
            perm.extend((c * NPC + nodes).tolist())
            er = rel[nodes].ravel()                            # relation per edge
            ei = inv[nodes].ravel()                            # compact nbr id
            ej = np.repeat(np.arange(TN), S)                   # local node idx
            slots_idx = np.zeros(NSLOT, dtype=np.int32)
            sel = np.zeros((128, NSLOT), dtype=fp8)            # [pos, r*128+node]
            for r in range(R1):
                m = er == r
                us, js = ei[m], ej[m]
                # group by node
                o = np.argsort(js, kind="stable")
                us, js = us[o], js[o]
                k = len(js)
                # forced pairs to fit 128 slots: pair same-node duplicates
                entries = []        # (row_id, node)
                if k > P:
                    need_pairs = k - P
                    bynode = {}
                    for u, j in zip(us.tolist(), js.tolist()):
                        bynode.setdefault(j, []).append(u)
                    items = sorted(bynode.items(),
                                   key=lambda kv: -len(kv[1]))
                    for j, ulist in items:
                        while need_pairs > 0 and len(ulist) >= 2:
                            u1, u2 = ulist.pop(), ulist.pop()
                            pair_rows.append((u1, u2))
                            entries.append((nrows + len(pair_rows) - 1, j))
                            need_pairs -= 1
                    assert need_pairs == 0, "pairing failed"
                    for j, ulist in items:
                        for u in ulist:
                            entries.append((int(u), j))
                else:
                    entries = list(zip(us.tolist(), js.tolist()))
                assert len(entries) <= P
                for pos, (row_id, j) in enumerate(entries):
                    slots_idx[r * P + pos] = row_id
                    sel[pos, r * 128 + j] = 1.0
            sel_all[t] = sel
            gsplit = tile_gsplit(t)
            idx_all[t] = np.tile(
                _wrap_idx(slots_idx.astype(np.int16), gsplit), (8, 1))
        npair = len(pair_rows)
        assert nrows + npair <= UMAX, (nrows, npair)
        emb_c = np.zeros((UMAX, 2 * D), dtype=fp8)
        emb_c[:U] = rows[0]
        if npair:
            pr = np.array(pair_rows, dtype=np.int64)
            emb_c[U:U + npair, :D] = eh[pr[:, 0]]
            emb_c[U:U + npair, D:] = eh[pr[:, 1]]
        in_maps.append({
            "emb": emb_c,
            "wsb": W_sb,
            "w8h": W8h,
            "w8l": W8l,
            "idx": np.ascontiguousarray(idx_all.reshape(NTILES * 128, IDXW)),
            "sel": np.ascontiguousarray(sel_all.reshape(NTILES * 128, NSLOT)),
        })
        perms.append(np.array(perm, dtype=np.int64))

    return in_maps, perms


# ---------------------------------------------------------------------------
# Numpy emulation (precision-faithful) for validation
# ---------------------------------------------------------------------------

def emulate_core(in_map):
    emb = in_map["emb"]                                        # [UMAX, 2D] fp8
    wsb = in_map["wsb"].reshape(128, RB, 2, D)                 # [p, j, c, o]
    w8h = in_map["w8h"].reshape(128, R8, 2, D).astype(np.float32)
    w8l = in_map["w8l"].reshape(128, R8, 2, D).astype(np.float32)
    idx = in_map["idx"].reshape(NTILES, 128, IDXW)
    sel = in_map["sel"].reshape(NTILES, 128, NSLOT)
    xfull = emb.astype(np.float32)
    xsum = xfull[:, :D] + xfull[:, D:]                         # plane0 + plane1
    out = np.zeros((NPC, D), dtype=np.float32)
    for t in range(NTILES):
        gsplit = tile_gsplit(t)
        parts, col = [], 0
        for a, b in zip(gsplit, gsplit[1:]):
            w = (b - a) * P // 16
            parts.append(idx[t, :16, col:col + w].T.reshape((b - a) * P))
            col += w
        slots_idx = np.concatenate(parts)                      # unwrap
        X = xsum[slots_idx]                                    # [NSLOT, D]
        out_acc = np.zeros((128, D), dtype=np.float32)
        for r in range(R1):
            selr = sel[t][:, r * 128:(r + 1) * 128].astype(np.float32)
            aggT = X[r * P:(r + 1) * P].T @ selr               # [D, 128] f32
            if r in CIDX:
                j = CIDX[r]
                hi = aggT.astype(fp8).astype(np.float32)
                res = (aggT - hi).astype(fp8e5).astype(np.float32)
                for cc in range(2):
                    hc = hi[cc * 128:(cc + 1) * 128]
                    rc = res[cc * 128:(cc + 1) * 128]
                    out_acc += hc.T @ (w8h[:, j, cc, :] + w8l[:, j, cc, :])
                    out_acc += rc.T @ w8h[:, j, cc, :]
            else:
                j = BIDX[r]
                aggTb = aggT.astype(bf16).astype(np.float32)   # evac to bf16
                for cc in range(2):
                    out_acc += aggTb[cc * 128:(cc + 1) * 128].T @ \
                        wsb[:, j, cc, :].astype(np.float32)
        out[t * TN:(t + 1) * TN] = np.maximum(out_acc, 0.0).astype(
            bf16).astype(np.float32)
    return out


def emulate(emb_table, weights, neighbors, relations):
    in_maps, perms = prep(emb_table, weights, neighbors, relations)
    full = np.zeros((N, D), dtype=np.float32)
    for c in range(NCORES):
        full[perms[c]] = emulate_core(in_maps[c])
    return full


# ---------------------------------------------------------------------------
# Bass program
# ---------------------------------------------------------------------------

def build_program():
    import concourse.bacc as bacc
    import concourse.tile as tile
    import concourse.mybir as mybir

    nc = bacc.Bacc(
        "TRN2", target_bir_lowering=False, debug=False,
        num_devices=NCORES,
    )
    BF = mybir.dt.bfloat16
    F32 = mybir.dt.float32
    I16 = mybir.dt.int16
    F8 = mybir.dt.float8e4
    F8E5 = mybir.dt.float8e5
    DR = mybir.MatmulPerfMode.DoubleRow

    emb = nc.dram_tensor("emb", [UMAX, 2 * D], F8, kind="ExternalInput").ap()
    wsb = nc.dram_tensor("wsb", [128, RB * 2 * D], BF, kind="ExternalInput").ap()
    w8h = nc.dram_tensor("w8h", [128, R8 * 2 * D], F8, kind="ExternalInput").ap()
    w8l = nc.dram_tensor("w8l", [128, R8 * 2 * D], F8E5, kind="ExternalInput").ap()
    idx = nc.dram_tensor("idx", [NTILES * 128, IDXW], I16, kind="ExternalInput").ap()
    sel = nc.dram_tensor("sel", [NTILES * 128, NSLOT], F8,
                         kind="ExternalInput").ap()
    out = nc.dram_tensor("out", [NPC, D], BF, kind="ExternalOutput").ap()

    Relu = mybir.ActivationFunctionType.Relu

    with tile.TileContext(nc) as tc:
        with (
            tc.tile_pool(name="wpool", bufs=1) as wpool,
            tc.tile_pool(name="xpool0", bufs=1) as xpool0,
            tc.tile_pool(name="xpool", bufs=PF + 1) as xpool,
            tc.tile_pool(name="selpool0", bufs=1) as selpool0,
            tc.tile_pool(name="selpool", bufs=PF + 1) as selpool,
            tc.tile_pool(name="idxpool", bufs=NTILES) as idxpool,
            tc.tile_pool(name="ypool", bufs=LAG + 3) as ypool,
            tc.tile_pool(name="ypool8", bufs=LAG + 3) as ypool8,
            tc.tile_pool(name="opool", bufs=2) as opool,
            tc.tile_pool(name="wupool", bufs=1) as wupool,
            tc.tile_pool(name="psy", bufs=SKEW + 2, space="PSUM") as psy,
            tc.tile_pool(name="pso", bufs=2, space="PSUM") as pso,
        ):
            wt = wpool.tile([128, RB * 2 * D], BF)
            w8ht = wpool.tile([128, R8 * 2 * D], F8)
            w8lt = wpool.tile([128, R8 * 2 * D], F8E5)

            def load_w(j0, j1):
                # packed bf16 weights (BF_RELS order)
                a, b = j0 * 2 * D, j1 * 2 * D
                nc.sync.dma_start(out=wt[:, a:b], in_=wsb[:, a:b])

            def load_w8(j0, j1, which):
                a, b = j0 * 2 * D, j1 * 2 * D
                if which == 0:
                    nc.sync.dma_start(out=w8ht[:, a:b], in_=w8h[:, a:b])
                else:
                    nc.sync.dma_start(out=w8lt[:, a:b], in_=w8l[:, a:b])

            # PE warmup: ramp the clock while the first gather lands.
            wu = wupool.tile([128, 128], BF)
            nc.vector.memset(wu[:], 0.0)
            wups = pso.tile([128, D], F32, name="outp")
            for _ in range(WARMUP):
                nc.tensor.matmul(out=wups[:, :128], lhsT=wu[:], rhs=wu[:],
                                 start=True, stop=True)

            # per-tile state: xs[t] = list of (g_lo, g_hi, tile);
            # sels[t] = list of (r_lo, r_hi, tile)
            xs, sels = {}, {}

            def pre_idx(t):
                idx_t = idxpool.tile([128, IDXW], I16, name="idx_t")
                nc.sync.dma_start(
                    out=idx_t[:], in_=idx[t * 128:(t + 1) * 128, :])
                return idx_t

            def gather_seg(t, idx_t, gi, a, b):
                n = (b - a) * P
                name = f"xt{gi}" if t == 0 else f"x{gi}"
                pool = xpool0 if t == 0 else xpool
                xt = pool.tile([128, b - a, 2 * D], F8, name=name)
                col = a * P // 16
                nc.gpsimd.dma_gather(
                    out_ap=xt[:],
                    in_ap=emb,
                    idxs_ap=idx_t[:, col:col + n // 16],
                    num_idxs=n,
                    num_idxs_reg=n,
                    elem_size=2 * D,
                    transpose=False,
                    single_packet=False,
                )
                xs.setdefault(t, []).append((a, b, xt))

            def load_sel(t, r0, r1, name, pool):
                st = pool.tile([128, (r1 - r0) * 128], F8, name=name)
                nc.sync.dma_start(
                    out=st[:],
                    in_=sel[t * 128:(t + 1) * 128, r0 * 128:r1 * 128])
                sels.setdefault(t, []).append((r0, r1, st))

            idxts = {}

            def prefetch(t):
                if t >= NTILES:
                    return
                idx_t = idxts[t]
                gs = tile_gsplit(t)
                for gi, (a, b) in enumerate(zip(gs, gs[1:])):
                    gather_seg(t, idx_t, gi, a, b)
                load_sel(t, 0, R1, "sel_t", selpool)

            def lookup(lst, r):
                for lo, hi, tl in lst:
                    if lo <= r < hi:
                        return lo, tl
                raise KeyError(r)

            # Startup orchestration: tile 0 finely segmented; W interleaved in
            # small chunks so the serial DMA engine feeds aggs, stage-B, and
            # the next tiles' gathers roughly in demand order.
            for _t in range(NTILES):
                idxts[_t] = pre_idx(_t)
            idx0 = idxts[0]

            # W loads in pair-demand order, popped between startup DMAs
            def pair_rels(k):
                return [r for r in (2 * k, 2 * k + 1) if r < R1]

            wload_ops = []
            for k in range(NPAIR):
                rs = pair_rels(k)
                if k in CONV_PAIRS:
                    j = CIDX[rs[0]]
                    wload_ops.append((load_w8, (j, j + len(rs), 0)))
                    wload_ops.append((load_w8, (j, j + len(rs), 1)))
                else:
                    j = BIDX[rs[0]]
                    wload_ops.append((load_w, (j, j + len(rs))))

            def pop_w(n):
                for _ in range(n):
                    if wload_ops:
                        f, args = wload_ops.pop(0)
                        f(*args)

            gather_seg(0, idx0, 0, *GSPLIT0[0:2])
            load_sel(0, 0, 6, "sel_a", selpool0)
            pop_w(1)
            gather_seg(0, idx0, 1, *GSPLIT0[1:3])
            pop_w(2)
            load_sel(0, 6, R1, "sel_b", selpool0)
            gather_seg(0, idx0, 2, *GSPLIT0[2:4])
            pop_w(3)
            gather_seg(0, idx0, 3, *GSPLIT0[3:5])
            pop_w(3)
            gather_seg(0, idx0, 4, *GSPLIT0[4:6])
            pop_w(3)
            prefetch(1)
            pop_w(4)
            prefetch(2)
            pop_w(4)
            prefetch(3)
            pop_w(len(wload_ops))

            # Unified pair stream across all tiles: agg(g) runs SKEW pairs
            # ahead of stage-B(g); tile boundaries vanish (the next tile's
            # aggs fill the previous tile's stage-B drain window).
            NPT = NTILES * NPAIR
            cur, outps, ys = {}, {}, {}
            nextq = [0]

            def emit_stage_b():
                q = nextq[0]
                nextq[0] += 1
                qt, qk = q // NPAIR, q % NPAIR
                if qk == 0:
                    outps[qt] = pso.tile([128, D], F32, name="outp")
                outp = outps[qt]
                ysq = ys.pop(q)
                if qk in CONV_PAIRS:
                    ysbh, ysbr = ysq
                    for u, r in enumerate(pair_rels(qk)):
                        j = CIDX[r]
                        wh = w8ht[:, j * 2 * D:(j + 1) * 2 * D].rearrange(
                            "p (c o) -> p c o", c=2)
                        wl = w8lt[:, j * 2 * D:(j + 1) * 2 * D].rearrange(
                            "p (c o) -> p c o", c=2)
                        for lhsT, rhs in (
                            (ysbh, wh),
                            (ysbh, wl),
                            (ysbr, wh),
                        ):
                            nc.tensor.matmul(
                                out=outp[:],
                                lhsT=lhsT[:, u, :, :],
                                rhs=rhs,
                                start=False, stop=False,
                                perf_mode=DR,
                            )
                else:
                    for u, r in enumerate(pair_rels(qk)):
                        for cc in range(2):
                            b = BIDX[r] * 2 + cc
                            nc.tensor.matmul(
                                out=outp[:],
                                lhsT=ysq[:, u, cc, :],
                                rhs=wt[:, b * D:(b + 1) * D],
                                start=(r == 0 and cc == 0),
                                stop=(r == R1 - 1 and cc == 1),
                            )
                if qk == NPAIR - 1:
                    outp = outps.pop(qt)
                    osb = opool.tile([128, D], BF)
                    nc.scalar.activation(out=osb[:], in_=outp[:], func=Relu)
                    nc.sync.dma_start(
                        out=out[qt * TN:(qt + 1) * TN, :], in_=osb[:])

            for g in range(NPT):
                t, k = g // NPAIR, g % NPAIR
                if k == 0:
                    cur[t] = (xs.pop(t), sels.pop(t))
                xlist, slist = cur[t]
                rs = pair_rels(k)
                yp = psy.tile([128, 2, 2, 128], F32, name="yp")
                for u, r in enumerate(rs):
                    slo, st = lookup(slist, r)
                    rhs = st[:, (r - slo) * 128:(r - slo + 1) * 128] \
                        .rearrange("p (u n) -> p u n", u=1) \
                        .broadcast_to([128, 2, 128])
                    glo, xt = lookup(xlist, r)
                    xr = xt[:, r - glo, :].rearrange(
                        "p (u d) -> p u d", u=2)
                    for cc in range(2):
                        nc.tensor.matmul(
                            out=yp[:, u, cc, :],
                            lhsT=xr[:, :, cc * 128:(cc + 1) * 128],
                            rhs=rhs,
                            start=True, stop=True,
                            perf_mode=DR,
                        )
                nu = len(rs)
                if k in CONV_PAIRS:
                    ysbh = ypool8.tile([128, 2, 2, 128], F8, name="ysbh")
                    ysbr = ypool8.tile([128, 2, 2, 128], F8E5, name="ysbr")
                    nc.scalar.copy(out=ysbh[:, :nu], in_=yp[:, :nu])
                    nc.vector.tensor_sub(
                        ysbr[:, :nu], yp[:, :nu], ysbh[:, :nu])
                    ys[g] = (ysbh, ysbr)
                else:
                    ysb = ypool.tile([128, 2, 2, 128], BF, name="ysb")
                    if g % 2 == 0:
                        nc.vector.tensor_copy(out=ysb[:, :nu], in_=yp[:, :nu])
                    else:
                        nc.scalar.copy(out=ysb[:, :nu], in_=yp[:, :nu])
                    ys[g] = ysb
                if k == NPAIR - 1:
                    prefetch(t + PF + 1)
                lag = LAG if g < NPT - 2 * NPAIR else LAG_END
                while nextq[0] <= g - lag:
                    emit_stage_b()
            while nextq[0] < NPT:
                emit_stage_b()

    nc.compile()
    return nc


_NC_CACHE = []


def _get_program():
    if not _NC_CACHE:
        _NC_CACHE.append(build_program())
    return _NC_CACHE[0]


# ---------------------------------------------------------------------------
# Entry point
# ---------------------------------------------------------------------------

def kernel(emb_table, weights, neighbors, relations):
    from concourse import bass_utils

    in_maps, perms = prep(emb_table, weights, neighbors, relations)
    nc = _get_program()
    res = bass_utils.run_bass_kernel_spmd(
        nc, in_maps, core_ids=list(range(NCORES)),
    )
    full = np.zeros((N, D), dtype=np.float32)
    for c in range(NCORES):
        full[perms[c]] = np.asarray(res.results[c]["out"]).astype(np.float32)
    return full


# revision 63
# speedup vs baseline: 1.1261x; 1.0057x over previous
"""Trainium2 Bass kernel for LoopRelationalGraphConvolution.

Math (matches the jax reference):
    out[n] = relu( SCALE * sum_s  W[rel[n,s]] @ emb[neighbors[n,s]] )
    SCALE  = 1000 / (R1 * S)      (folds the mean over S and the /R1 * 1000)

Design (8 NeuronCores, data-parallel over the 8192-node batch), aggregate-first:
  Each core owns 1024 nodes in 8 node-tiles of 128.  Per tile, relation r's
  edges occupy gather group r (slots 128r..128r+127).  Buckets with more than
  128 edges are reduced by pairing two edges of the SAME (node, relation):
  the pair becomes one 512B table row [fp8_hi(u) | fp8_hi(v)]; normal rows are
  [fp8_hi(emb) | fp8_lo(emb - hi)].  Per tile the device kernel:
    1. dma_gather (transpose=False): slot i lands on partition i%128, group
       i//128; each slot's 512B row is contiguous in the free dim.
    2. aggregation matmuls (fp8 DoubleRow): per (r, d-chunk c) one DR matmul
       contracts k-tiles (bytes 0:256, bytes 256:512) against a stride-0
       broadcast of the 0/1 sel matrix:
         aggT[d, node] = sum_slots (plane0 + plane1)[d] * sel[slot, node]
       exact f32 PSUM accumulation; 128-col DR output = 26.7ns each.
    3. evac aggT (two relations share one PSUM bank) -> SBUF bf16.
    4. stage-B matmuls (bf16): out[node, o] += aggT[:, c, :]^T @ W[r, c]
       accumulated over all (r, c) in PSUM.
    5. relu on PSUM->SBUF evacuation (bf16), DMA node rows to DRAM.
  The device program is static and identical across cores (SPMD); all data
  dependence lives in the index / selection arrays.  Host post-step inverse-
  permutes rows back to the original node order.
"""

import numpy as np
import ml_dtypes

bf16 = ml_dtypes.bfloat16
fp8 = ml_dtypes.float8_e4m3
fp8e5 = ml_dtypes.float8_e5m2

# Problem constants (hardcoded per contract).
V = 100000
D = 256
R1 = 33          # relations incl. self-loop
N = 8192
S = 32
NCORES = 8
NPC = N // NCORES          # 1024 nodes per core
NTILES = 8                 # node-tiles per core
TN = 128                   # nodes per tile
P = 128
NSLOT = R1 * P             # 4224 edge slots per tile
# gather segments (group boundaries): tile 0 split fine for fast startup,
# last tile ends with a small segment to shorten the data-dependent tail
GSPLIT0 = [0, 2, 6, 14, 24, R1]
GSPLIT = [0, 16, R1]
GSPLITL = [0, 16, 28, R1]


def tile_gsplit(t):
    if t == 0:
        return GSPLIT0
    if t == NTILES - 1:
        return GSPLITL
    return GSPLIT
IDXW = NSLOT // 16         # 264 int16 idx columns (16-partition wrap)
UMAX = 32768               # compacted per-core embedding rows (int16 limit)
SCALE = 1000.0 / (R1 * S)

# Software-pipeline skew (in relation PAIRS) between aggregation and stage-B,
# so the PE never stalls on the PSUM->SBUF copy.
SKEW = 4                   # agg -> evac PSUM depth (PSUM-bank limited)
LAG = 4                    # agg -> stage-B lag in pairs (SBUF-buffered)
LAG_END = 4                # shrunken lag near the stream end (short tail)
NPAIR = (R1 + 1) // 2      # 17 relation pairs per tile (last is a single)
CONV_PAIRS = frozenset((9,))  # fp8 DoubleRow stage-B pairs
POOL_EVAC = frozenset()    # unconverted evacs routed to GPSIMD
CONV_RELS = tuple(sorted(r for k in CONV_PAIRS for r in (2 * k, 2 * k + 1)))
BF_RELS = tuple(r for r in range(R1) if r not in CONV_RELS)
CIDX = {r: i for i, r in enumerate(CONV_RELS)}
BIDX = {r: i for i, r in enumerate(BF_RELS)}
R8 = len(CONV_RELS)        # 14 relations with fp8 stage-B (W as fp8 hi+lo)
RB = len(BF_RELS)          # 19 relations with bf16 stage-B
PF = 3      # tile prefetch depth
WARMUP = 54  # PE warmup matmuls (p-state ramp + cover first gather latency)


# ---------------------------------------------------------------------------
# Host-side preparation
# ---------------------------------------------------------------------------

def _balance_tiles(hist):
    """Assign NPC nodes to NTILES tiles (TN nodes each), minimizing the max
    per-(tile, relation) SLOT need, where a (node, rel) multiplicity k can be
    compressed to ceil(k/2) slots by same-bucket pairing.  hist: [NPC, R1].
    Returns tiles (lists of node ids)."""
    order = np.argsort(-hist.max(axis=1), kind="stable")
    loads = np.zeros((NTILES, R1), dtype=np.int64)     # raw edge counts
    pav = np.zeros((NTILES, R1), dtype=np.int64)       # pairs available
    counts = np.zeros(NTILES, dtype=np.int64)
    tiles = [[] for _ in range(NTILES)]
    for n in order:
        h = hist[n]
        hp = h // 2
        best_t, best_key = -1, None
        for t in range(NTILES):
            if counts[t] >= TN:
                continue
            nl = loads[t] + h
            np_ = pav[t] + hp
            need = nl - np.minimum(np_, np.maximum(nl - P, 0))
            key = (int(need.max()), int(nl.max()), int(nl.sum()))
            if best_key is None or key < best_key:
                best_key, best_t = key, t
        tiles[best_t].append(int(n))
        loads[best_t] += h
        pav[best_t] += hp
        counts[best_t] += 1
    need = loads - np.minimum(pav, np.maximum(loads - P, 0))
    assert need.max() <= P, f"balance failed: max need {need.max()}"
    return tiles


def _wrap_idx(slots_idx, gsplit):
    """Per-segment 16-partition wrap: segment [a, b) of the slot-index vector
    becomes idx[i%16, a//16 + i//16]."""
    parts = []
    for a, b in zip(gsplit, gsplit[1:]):
        a, b = a * P, b * P
        parts.append(slots_idx[a:b].reshape((b - a) // 16, 16).T)
    return np.concatenate(parts, axis=1)        # [16, IDXW]


def prep(emb_table, weights, neighbors, relations):
    """Build per-core device arrays. Returns (in_maps, perms)."""
    emb_f = np.asarray(emb_table, dtype=np.float32)
    w = np.asarray(weights, dtype=np.float32) * SCALE         # [R1, D_out, D_in]
    # W_sb[p, (r*2 + c)*D + o] = w[r, o, c*128+p]
    w_rdo = np.ascontiguousarray(w.transpose(0, 2, 1))        # [r, d, o]
    W_pc = np.ascontiguousarray(
        w_rdo.reshape(R1, 2, 128, D).transpose(2, 0, 1, 3))   # [p, r, c, o]
    W_sb = np.ascontiguousarray(
        W_pc[:, list(BF_RELS)]).reshape(128, RB * 2 * D).astype(bf16)
    w8 = np.ascontiguousarray(W_pc[:, list(CONV_RELS)]).astype(np.float32)
    W8h = w8.astype(fp8)
    W8l = (w8 - W8h.astype(np.float32)).astype(fp8e5)
    W8h = np.ascontiguousarray(W8h.reshape(128, R8 * 2 * D))
    W8l = np.ascontiguousarray(W8l.reshape(128, R8 * 2 * D))

    neighbors = np.asarray(neighbors).astype(np.int64)
    relations = np.asarray(relations).astype(np.int64)

    in_maps, perms = [], []
    for c in range(NCORES):
        nb = neighbors[c * NPC:(c + 1) * NPC]                 # [NPC, S]
        rel = relations[c * NPC:(c + 1) * NPC]
        uniq, inv = np.unique(nb.ravel(), return_inverse=True)
        inv = inv.reshape(nb.shape).astype(np.int64)
        U = len(uniq)
        eh = emb_f[uniq].astype(fp8)                          # [U, D] hi
        el = (emb_f[uniq] - eh.astype(np.float32)).astype(fp8)  # lo
        rows = [np.concatenate([eh, el], axis=1)]             # [U, 2D]
        nrows = U

        hist = np.zeros((NPC, R1), dtype=np.int64)
        np.add.at(hist, (np.repeat(np.arange(NPC), S), rel.ravel()), 1)
        tiles = _balance_tiles(hist)

        idx_all = np.zeros((NTILES, 128, IDXW), dtype=np.int16)
        sel_all = np.zeros((NTILES, 128, NSLOT), dtype=fp8)
        pair_rows = []
        perm = []
        for t, nodes in enumerate(tiles):
            nodes = np.array(nodes, dtype=np.int64)
            assert len(nodes) == TN
            perm.extend((c * NPC + nodes).tolist())
            er = rel[nodes].ravel()                            # relation per edge
            ei = inv[nodes].ravel()                            # compact nbr id
            ej = np.repeat(np.arange(TN), S)                   # local node idx
            slots_idx = np.zeros(NSLOT, dtype=np.int32)
            sel = np.zeros((128, NSLOT), dtype=fp8)            # [pos, r*128+node]
            for r in range(R1):
                m = er == r
                us, js = ei[m], ej[m]
                # group by node
                o = np.argsort(js, kind="stable")
                us, js = us[o], js[o]
                k = len(js)
                # forced pairs to fit 128 slots: pair same-node duplicates
                entries = []        # (row_id, node)
                if k > P:
                    need_pairs = k - P
                    bynode = {}
                    for u, j in zip(us.tolist(), js.tolist()):
                        bynode.setdefault(j, []).append(u)
                    items = sorted(bynode.items(),
                                   key=lambda kv: -len(kv[1]))
                    for j, ulist in items:
                        while need_pairs > 0 and len(ulist) >= 2:
                            u1, u2 = ulist.pop(), ulist.pop()
                            pair_rows.append((u1, u2))
                            entries.append((nrows + len(pair_rows) - 1, j))
                            need_pairs -= 1
                    assert need_pairs == 0, "pairing failed"
                    for j, ulist in items:
                        for u in ulist:
                            entries.append((int(u), j))
                else:
                    entries = list(zip(us.tolist(), js.tolist()))
                assert len(entries) <= P
                for pos, (row_id, j) in enumerate(entries):
                    slots_idx[r * P + pos] = row_id
                    sel[pos, r * 128 + j] = 1.0
            sel_all[t] = sel
            gsplit = tile_gsplit(t)
            idx_all[t] = np.tile(
                _wrap_idx(slots_idx.astype(np.int16), gsplit), (8, 1))
        npair = len(pair_rows)
        assert nrows + npair <= UMAX, (nrows, npair)
        emb_c = np.zeros((UMAX, 2 * D), dtype=fp8)
        emb_c[:U] = rows[0]
        if npair:
            pr = np.array(pair_rows, dtype=np.int64)
            emb_c[U:U + npair, :D] = eh[pr[:, 0]]
            emb_c[U:U + npair, D:] = eh[pr[:, 1]]
        in_maps.append({
            "emb": emb_c,
            "wsb": W_sb,
            "w8h": W8h,
            "w8l": W8l,
            "idx": np.ascontiguousarray(idx_all.reshape(NTILES * 128, IDXW)),
            "sel": np.ascontiguousarray(sel_all.reshape(NTILES * 128, NSLOT)),
        })
        perms.append(np.array(perm, dtype=np.int64))

    return in_maps, perms


# ---------------------------------------------------------------------------
# Numpy emulation (precision-faithful) for validation
# ---------------------------------------------------------------------------

def emulate_core(in_map):
    emb = in_map["emb"]                                        # [UMAX, 2D] fp8
    wsb = in_map["wsb"].reshape(128, RB, 2, D)                 # [p, j, c, o]
    w8h = in_map["w8h"].reshape(128, R8, 2, D).astype(np.float32)
    w8l = in_map["w8l"].reshape(128, R8, 2, D).astype(np.float32)
    idx = in_map["idx"].reshape(NTILES, 128, IDXW)
    sel = in_map["sel"].reshape(NTILES, 128, NSLOT)
    xfull = emb.astype(np.float32)
    xsum = xfull[:, :D] + xfull[:, D:]                         # plane0 + plane1
    out = np.zeros((NPC, D), dtype=np.float32)
    for t in range(NTILES):
        gsplit = tile_gsplit(t)
        parts, col = [], 0
        for a, b in zip(gsplit, gsplit[1:]):
            w = (b - a) * P // 16
            parts.append(idx[t, :16, col:col + w].T.reshape((b - a) * P))
            col += w
        slots_idx = np.concatenate(parts)                      # unwrap
        X = xsum[slots_idx]                                    # [NSLOT, D]
        out_acc = np.zeros((128, D), dtype=np.float32)
        for r in range(R1):
            selr = sel[t][:, r * 128:(r + 1) * 128].astype(np.float32)
            aggT = X[r * P:(r + 1) * P].T @ selr               # [D, 128] f32
            if r in CIDX:
                j = CIDX[r]
                hi = aggT.astype(fp8).astype(np.float32)
                res = (aggT - hi).astype(fp8e5).astype(np.float32)
                for cc in range(2):
                    hc = hi[cc * 128:(cc + 1) * 128]
                    rc = res[cc * 128:(cc + 1) * 128]
                    out_acc += hc.T @ (w8h[:, j, cc, :] + w8l[:, j, cc, :])
                    out_acc += rc.T @ w8h[:, j, cc, :]
            else:
                j = BIDX[r]
                aggTb = aggT.astype(bf16).astype(np.float32)   # evac to bf16
                for cc in range(2):
                    out_acc += aggTb[cc * 128:(cc + 1) * 128].T @ \
                        wsb[:, j, cc, :].astype(np.float32)
        out[t * TN:(t + 1) * TN] = np.maximum(out_acc, 0.0).astype(
            bf16).astype(np.float32)
    return out


def emulate(emb_table, weights, neighbors, relations):
    in_maps, perms = prep(emb_table, weights, neighbors, relations)
    full = np.zeros((N, D), dtype=np.float32)
    for c in range(NCORES):
        full[perms[c]] = emulate_core(in_maps[c])
    return full


# ---------------------------------------------------------------------------
# Bass program
# ---------------------------------------------------------------------------

def build_program():
    import concourse.bacc as bacc
    import concourse.tile as tile
    import concourse.mybir as mybir

    nc = bacc.Bacc(
        "TRN2", target_bir_lowering=False, debug=False,
        num_devices=NCORES,
    )
    BF = mybir.dt.bfloat16
    F32 = mybir.dt.float32
    I16 = mybir.dt.int16
    F8 = mybir.dt.float8e4
    F8E5 = mybir.dt.float8e5
    DR = mybir.MatmulPerfMode.DoubleRow

    emb = nc.dram_tensor("emb", [UMAX, 2 * D], F8, kind="ExternalInput").ap()
    wsb = nc.dram_tensor("wsb", [128, RB * 2 * D], BF, kind="ExternalInput").ap()
    w8h = nc.dram_tensor("w8h", [128, R8 * 2 * D], F8, kind="ExternalInput").ap()
    w8l = nc.dram_tensor("w8l", [128, R8 * 2 * D], F8E5, kind="ExternalInput").ap()
    idx = nc.dram_tensor("idx", [NTILES * 128, IDXW], I16, kind="ExternalInput").ap()
    sel = nc.dram_tensor("sel", [NTILES * 128, NSLOT], F8,
                         kind="ExternalInput").ap()
    out = nc.dram_tensor("out", [NPC, D], BF, kind="ExternalOutput").ap()

    Relu = mybir.ActivationFunctionType.Relu

    with tile.TileContext(nc) as tc:
        with (
            tc.tile_pool(name="wpool", bufs=1) as wpool,
            tc.tile_pool(name="xpool0", bufs=1) as xpool0,
            tc.tile_pool(name="xpool", bufs=PF + 1) as xpool,
            tc.tile_pool(name="selpool0", bufs=1) as selpool0,
            tc.tile_pool(name="selpool", bufs=PF + 1) as selpool,
            tc.tile_pool(name="idxpool", bufs=PF + 1) as idxpool,
            tc.tile_pool(name="ypool", bufs=LAG + 3) as ypool,
            tc.tile_pool(name="ypool8", bufs=LAG + 3) as ypool8,
            tc.tile_pool(name="opool", bufs=2) as opool,
            tc.tile_pool(name="wupool", bufs=1) as wupool,
            tc.tile_pool(name="psy", bufs=SKEW + 2, space="PSUM") as psy,
            tc.tile_pool(name="pso", bufs=2, space="PSUM") as pso,
        ):
            wt = wpool.tile([128, RB * 2 * D], BF)
            w8ht = wpool.tile([128, R8 * 2 * D], F8)
            w8lt = wpool.tile([128, R8 * 2 * D], F8E5)

            def load_w(j0, j1):
                # packed bf16 weights (BF_RELS order)
                a, b = j0 * 2 * D, j1 * 2 * D
                nc.sync.dma_start(out=wt[:, a:b], in_=wsb[:, a:b])

            def load_w8(j0, j1, which):
                a, b = j0 * 2 * D, j1 * 2 * D
                if which == 0:
                    nc.sync.dma_start(out=w8ht[:, a:b], in_=w8h[:, a:b])
                else:
                    nc.sync.dma_start(out=w8lt[:, a:b], in_=w8l[:, a:b])

            # PE warmup: ramp the clock while the first gather lands.
            wu = wupool.tile([128, 128], BF)
            nc.vector.memset(wu[:], 0.0)
            wups = pso.tile([128, D], F32, name="outp")
            for _ in range(WARMUP):
                nc.tensor.matmul(out=wups[:, :128], lhsT=wu[:], rhs=wu[:],
                                 start=True, stop=True)

            # per-tile state: xs[t] = list of (g_lo, g_hi, tile);
            # sels[t] = list of (r_lo, r_hi, tile)
            xs, sels = {}, {}

            def pre_idx(t):
                idx_t = idxpool.tile([128, IDXW], I16, name="idx_t")
                nc.sync.dma_start(
                    out=idx_t[:], in_=idx[t * 128:(t + 1) * 128, :])
                return idx_t

            def gather_seg(t, idx_t, gi, a, b):
                n = (b - a) * P
                name = f"xt{gi}" if t == 0 else f"x{gi}"
                pool = xpool0 if t == 0 else xpool
                xt = pool.tile([128, b - a, 2 * D], F8, name=name)
                col = a * P // 16
                nc.gpsimd.dma_gather(
                    out_ap=xt[:],
                    in_ap=emb,
                    idxs_ap=idx_t[:, col:col + n // 16],
                    num_idxs=n,
                    num_idxs_reg=n,
                    elem_size=2 * D,
                    transpose=False,
                    single_packet=False,
                )
                xs.setdefault(t, []).append((a, b, xt))

            def load_sel(t, r0, r1, name, pool):
                st = pool.tile([128, (r1 - r0) * 128], F8, name=name)
                nc.sync.dma_start(
                    out=st[:],
                    in_=sel[t * 128:(t + 1) * 128, r0 * 128:r1 * 128])
                sels.setdefault(t, []).append((r0, r1, st))

            def prefetch(t):
                if t >= NTILES:
                    return
                idx_t = pre_idx(t)
                gs = tile_gsplit(t)
                for gi, (a, b) in enumerate(zip(gs, gs[1:])):
                    gather_seg(t, idx_t, gi, a, b)
                load_sel(t, 0, R1, "sel_t", selpool)

            def lookup(lst, r):
                for lo, hi, tl in lst:
                    if lo <= r < hi:
                        return lo, tl
                raise KeyError(r)

            # Startup orchestration: tile 0 finely segmented; W interleaved in
            # small chunks so the serial DMA engine feeds aggs, stage-B, and
            # the next tiles' gathers roughly in demand order.
            idx0 = pre_idx(0)

            # W loads in pair-demand order, popped between startup DMAs
            def pair_rels(k):
                return [r for r in (2 * k, 2 * k + 1) if r < R1]

            wload_ops = []
            for k in range(NPAIR):
                rs = pair_rels(k)
                if k in CONV_PAIRS:
                    j = CIDX[rs[0]]
                    wload_ops.append((load_w8, (j, j + len(rs), 0)))
                    wload_ops.append((load_w8, (j, j + len(rs), 1)))
                else:
                    j = BIDX[rs[0]]
                    wload_ops.append((load_w, (j, j + len(rs))))

            def pop_w(n):
                for _ in range(n):
                    if wload_ops:
                        f, args = wload_ops.pop(0)
                        f(*args)

            gather_seg(0, idx0, 0, *GSPLIT0[0:2])
            load_sel(0, 0, 6, "sel_a", selpool0)
            gather_seg(0, idx0, 1, *GSPLIT0[1:3])
            load_sel(0, 6, R1, "sel_b", selpool0)
            gather_seg(0, idx0, 2, *GSPLIT0[2:4])
            pop_w(3)
            pop_w(3)
            gather_seg(0, idx0, 3, *GSPLIT0[3:5])
            pop_w(3)
            gather_seg(0, idx0, 4, *GSPLIT0[4:6])
            pop_w(3)
            prefetch(1)
            pop_w(4)
            prefetch(2)
            pop_w(4)
            prefetch(3)
            pop_w(len(wload_ops))

            # Unified pair stream across all tiles: agg(g) runs SKEW pairs
            # ahead of stage-B(g); tile boundaries vanish (the next tile's
            # aggs fill the previous tile's stage-B drain window).
            NPT = NTILES * NPAIR
            cur, outps, ys = {}, {}, {}
            nextq = [0]

            def emit_stage_b():
                q = nextq[0]
                nextq[0] += 1
                qt, qk = q // NPAIR, q % NPAIR
                if qk == 0:
                    outps[qt] = pso.tile([128, D], F32, name="outp")
                outp = outps[qt]
                ysq = ys.pop(q)
                if qk in CONV_PAIRS:
                    ysbh, ysbr = ysq
                    for u, r in enumerate(pair_rels(qk)):
                        j = CIDX[r]
                        wh = w8ht[:, j * 2 * D:(j + 1) * 2 * D].rearrange(
                            "p (c o) -> p c o", c=2)
                        wl = w8lt[:, j * 2 * D:(j + 1) * 2 * D].rearrange(
                            "p (c o) -> p c o", c=2)
                        for lhsT, rhs in (
                            (ysbh, wh),
                            (ysbh, wl),
                            (ysbr, wh),
                        ):
                            nc.tensor.matmul(
                                out=outp[:],
                                lhsT=lhsT[:, u, :, :],
                                rhs=rhs,
                                start=False, stop=False,
                                perf_mode=DR,
                            )
                else:
                    for u, r in enumerate(pair_rels(qk)):
                        for cc in range(2):
                            b = BIDX[r] * 2 + cc
                            nc.tensor.matmul(
                                out=outp[:],
                                lhsT=ysq[:, u, cc, :],
                                rhs=wt[:, b * D:(b + 1) * D],
                                start=(r == 0 and cc == 0),
                                stop=(r == R1 - 1 and cc == 1),
                            )
                if qk == NPAIR - 1:
                    outp = outps.pop(qt)
                    osb = opool.tile([128, D], BF)
                    nc.scalar.activation(out=osb[:], in_=outp[:], func=Relu)
                    nc.sync.dma_start(
                        out=out[qt * TN:(qt + 1) * TN, :], in_=osb[:])

            for g in range(NPT):
                t, k = g // NPAIR, g % NPAIR
                if k == 0:
                    cur[t] = (xs.pop(t), sels.pop(t))
                xlist, slist = cur[t]
                rs = pair_rels(k)
                yp = psy.tile([128, 2, 2, 128], F32, name="yp")
                for u, r in enumerate(rs):
                    slo, st = lookup(slist, r)
                    rhs = st[:, (r - slo) * 128:(r - slo + 1) * 128] \
                        .rearrange("p (u n) -> p u n", u=1) \
                        .broadcast_to([128, 2, 128])
                    glo, xt = lookup(xlist, r)
                    xr = xt[:, r - glo, :].rearrange(
                        "p (u d) -> p u d", u=2)
                    for cc in range(2):
                        nc.tensor.matmul(
                            out=yp[:, u, cc, :],
                            lhsT=xr[:, :, cc * 128:(cc + 1) * 128],
                            rhs=rhs,
                            start=True, stop=True,
                            perf_mode=DR,
                        )
                nu = len(rs)
                if k in CONV_PAIRS:
                    ysbh = ypool8.tile([128, 2, 2, 128], F8, name="ysbh")
                    ysbr = ypool8.tile([128, 2, 2, 128], F8E5, name="ysbr")
                    nc.scalar.copy(out=ysbh[:, :nu], in_=yp[:, :nu])
                    nc.vector.tensor_sub(
                        ysbr[:, :nu], yp[:, :nu], ysbh[:, :nu])
                    ys[g] = (ysbh, ysbr)
                else:
                    ysb = ypool.tile([128, 2, 2, 128], BF, name="ysb")
                    if g % 2 == 0:
                        nc.vector.tensor_copy(out=ysb[:, :nu], in_=yp[:, :nu])
                    else:
                        nc.scalar.copy(out=ysb[:, :nu], in_=yp[:, :nu])
                    ys[g] = ysb
                if k == NPAIR - 1:
                    prefetch(t + PF + 1)
                lag = LAG if g < NPT - 2 * NPAIR else LAG_END
                while nextq[0] <= g - lag:
                    emit_stage_b()
            while nextq[0] < NPT:
                emit_stage_b()

    nc.compile()
    return nc


_NC_CACHE = []


def _get_program():
    if not _NC_CACHE:
        _NC_CACHE.append(build_program())
    return _NC_CACHE[0]


# ---------------------------------------------------------------------------
# Entry point
# ---------------------------------------------------------------------------

def kernel(emb_table, weights, neighbors, relations):
    from concourse import bass_utils

    in_maps, perms = prep(emb_table, weights, neighbors, relations)
    nc = _get_program()
    res = bass_utils.run_bass_kernel_spmd(
        nc, in_maps, core_ids=list(range(NCORES)),
    )
    full = np.zeros((N, D), dtype=np.float32)
    for c in range(NCORES):
        full[perms[c]] = np.asarray(res.results[c]["out"]).astype(np.float32)
    return full


# revision 66
# speedup vs baseline: 1.1935x; 1.0598x over previous
"""Trainium2 Bass kernel for LoopRelationalGraphConvolution.

Math (matches the jax reference):
    out[n] = relu( SCALE * sum_s  W[rel[n,s]] @ emb[neighbors[n,s]] )
    SCALE  = 1000 / (R1 * S)      (folds the mean over S and the /R1 * 1000)

Design (8 NeuronCores, data-parallel over the 8192-node batch), aggregate-first:
  Each core owns 1024 nodes in 8 node-tiles of 128.  Per tile, relation r's
  edges occupy gather group r (slots 128r..128r+127).  Buckets with more than
  128 edges are reduced by pairing two edges of the SAME (node, relation):
  the pair becomes one 512B table row [fp8_hi(u) | fp8_hi(v)]; normal rows are
  [fp8_hi(emb) | fp8_lo(emb - hi)].  Per tile the device kernel:
    1. dma_gather (transpose=False): slot i lands on partition i%128, group
       i//128; each slot's 512B row is contiguous in the free dim.
    2. aggregation matmuls (fp8 DoubleRow): per (r, d-chunk c) one DR matmul
       contracts k-tiles (bytes 0:256, bytes 256:512) against a stride-0
       broadcast of the 0/1 sel matrix:
         aggT[d, node] = sum_slots (plane0 + plane1)[d] * sel[slot, node]
       exact f32 PSUM accumulation; 128-col DR output = 26.7ns each.
    3. evac aggT (two relations share one PSUM bank) -> SBUF bf16.
    4. stage-B matmuls (bf16): out[node, o] += aggT[:, c, :]^T @ W[r, c]
       accumulated over all (r, c) in PSUM.
    5. relu on PSUM->SBUF evacuation (bf16), DMA node rows to DRAM.
  The device program is static and identical across cores (SPMD); all data
  dependence lives in the index / selection arrays.  Host post-step inverse-
  permutes rows back to the original node order.
"""

import numpy as np
import ml_dtypes

bf16 = ml_dtypes.bfloat16
fp8 = ml_dtypes.float8_e4m3
fp8e5 = ml_dtypes.float8_e5m2

# Problem constants (hardcoded per contract).
V = 100000
D = 256
R1 = 33          # relations incl. self-loop
N = 8192
S = 32
NCORES = 8
NPC = N // NCORES          # 1024 nodes per core
NTILES = 8                 # node-tiles per core
TN = 128                   # nodes per tile
P = 128
NSLOT = R1 * P             # 4224 edge slots per tile
# gather segments (group boundaries): tile 0 split fine for fast startup,
# last tile ends with a small segment to shorten the data-dependent tail
GSPLIT0 = [0, 2, 6, 14, 24, R1]
GSPLIT = [0, 16, R1]
GSPLITL = [0, 16, 28, R1]


def tile_gsplit(t):
    if t == 0:
        return GSPLIT0
    if t == NTILES - 1:
        return GSPLITL
    return GSPLIT
IDXW = NSLOT // 16         # 264 int16 idx columns (16-partition wrap)
UMAX = 32768               # compacted per-core embedding rows (int16 limit)
SCALE = 1000.0 / (R1 * S)

# Software-pipeline skew (in relation PAIRS) between aggregation and stage-B,
# so the PE never stalls on the PSUM->SBUF copy.
SKEW = 4                   # agg -> evac PSUM depth (PSUM-bank limited)
LAG = 4                    # agg -> stage-B lag in pairs (SBUF-buffered)
LAG_END = 4                # shrunken lag near the stream end (short tail)
NPAIR = (R1 + 1) // 2      # 17 relation pairs per tile (last is a single)
CONV_PAIRS = frozenset((3, 5, 9, 13))  # fp8 hi-only DoubleRow stage-B pairs
POOL_EVAC = frozenset()    # unconverted evacs routed to GPSIMD
CONV_RELS = tuple(sorted(r for k in CONV_PAIRS for r in (2 * k, 2 * k + 1)))
BF_RELS = tuple(r for r in range(R1) if r not in CONV_RELS)
CIDX = {r: i for i, r in enumerate(CONV_RELS)}
BIDX = {r: i for i, r in enumerate(BF_RELS)}
R8 = len(CONV_RELS)        # 14 relations with fp8 stage-B (W as fp8 hi+lo)
RB = len(BF_RELS)          # 19 relations with bf16 stage-B
PF = 3      # tile prefetch depth
WARMUP = 54  # PE warmup matmuls (p-state ramp + cover first gather latency)


# ---------------------------------------------------------------------------
# Host-side preparation
# ---------------------------------------------------------------------------

def _balance_tiles(hist):
    """Assign NPC nodes to NTILES tiles (TN nodes each), minimizing the max
    per-(tile, relation) SLOT need, where a (node, rel) multiplicity k can be
    compressed to ceil(k/2) slots by same-bucket pairing.  hist: [NPC, R1].
    Returns tiles (lists of node ids)."""
    order = np.argsort(-hist.max(axis=1), kind="stable")
    loads = np.zeros((NTILES, R1), dtype=np.int64)     # raw edge counts
    pav = np.zeros((NTILES, R1), dtype=np.int64)       # pairs available
    counts = np.zeros(NTILES, dtype=np.int64)
    tiles = [[] for _ in range(NTILES)]
    for n in order:
        h = hist[n]
        hp = h // 2
        best_t, best_key = -1, None
        for t in range(NTILES):
            if counts[t] >= TN:
                continue
            nl = loads[t] + h
            np_ = pav[t] + hp
            need = nl - np.minimum(np_, np.maximum(nl - P, 0))
            key = (int(need.max()), int(nl.max()), int(nl.sum()))
            if best_key is None or key < best_key:
                best_key, best_t = key, t
        tiles[best_t].append(int(n))
        loads[best_t] += h
        pav[best_t] += hp
        counts[best_t] += 1
    need = loads - np.minimum(pav, np.maximum(loads - P, 0))
    assert need.max() <= P, f"balance failed: max need {need.max()}"
    return tiles


def _wrap_idx(slots_idx, gsplit):
    """Per-segment 16-partition wrap: segment [a, b) of the slot-index vector
    becomes idx[i%16, a//16 + i//16]."""
    parts = []
    for a, b in zip(gsplit, gsplit[1:]):
        a, b = a * P, b * P
        parts.append(slots_idx[a:b].reshape((b - a) // 16, 16).T)
    return np.concatenate(parts, axis=1)        # [16, IDXW]


def prep(emb_table, weights, neighbors, relations):
    """Build per-core device arrays. Returns (in_maps, perms)."""
    emb_f = np.asarray(emb_table, dtype=np.float32)
    w = np.asarray(weights, dtype=np.float32) * SCALE         # [R1, D_out, D_in]
    # W_sb[p, (r*2 + c)*D + o] = w[r, o, c*128+p]
    w_rdo = np.ascontiguousarray(w.transpose(0, 2, 1))        # [r, d, o]
    W_pc = np.ascontiguousarray(
        w_rdo.reshape(R1, 2, 128, D).transpose(2, 0, 1, 3))   # [p, r, c, o]
    W_sb = np.ascontiguousarray(
        W_pc[:, list(BF_RELS)]).reshape(128, RB * 2 * D).astype(bf16)
    w8 = np.ascontiguousarray(W_pc[:, list(CONV_RELS)]).astype(np.float32)
    W8h = w8.astype(fp8)
    W8l = (w8 - W8h.astype(np.float32)).astype(fp8e5)
    W8h = np.ascontiguousarray(W8h.reshape(128, R8 * 2 * D))
    W8l = np.ascontiguousarray(W8l.reshape(128, R8 * 2 * D))

    neighbors = np.asarray(neighbors).astype(np.int64)
    relations = np.asarray(relations).astype(np.int64)

    in_maps, perms = [], []
    for c in range(NCORES):
        nb = neighbors[c * NPC:(c + 1) * NPC]                 # [NPC, S]
        rel = relations[c * NPC:(c + 1) * NPC]
        uniq, inv = np.unique(nb.ravel(), return_inverse=True)
        inv = inv.reshape(nb.shape).astype(np.int64)
        U = len(uniq)
        eh = emb_f[uniq].astype(fp8)                          # [U, D] hi
        el = (emb_f[uniq] - eh.astype(np.float32)).astype(fp8)  # lo
        rows = [np.concatenate([eh, el], axis=1)]             # [U, 2D]
        nrows = U

        hist = np.zeros((NPC, R1), dtype=np.int64)
        np.add.at(hist, (np.repeat(np.arange(NPC), S), rel.ravel()), 1)
        tiles = _balance_tiles(hist)

        idx_all = np.zeros((NTILES, 128, IDXW), dtype=np.int16)
        sel_all = np.zeros((NTILES, 128, NSLOT), dtype=fp8)
        pair_rows = []
        perm = []
        for t, nodes in enumerate(tiles):
            nodes = np.array(nodes, dtype=np.int64)
            assert len(nodes) == TN
            perm.extend((c * NPC + nodes).tolist())
            er = rel[nodes].ravel()                            # relation per edge
            ei = inv[nodes].ravel()                            # compact nbr id
            ej = np.repeat(np.arange(TN), S)                   # local node idx
            slots_idx = np.zeros(NSLOT, dtype=np.int32)
            sel = np.zeros((128, NSLOT), dtype=fp8)            # [pos, r*128+node]
            for r in range(R1):
                m = er == r
                us, js = ei[m], ej[m]
                # group by node
                o = np.argsort(js, kind="stable")
                us, js = us[o], js[o]
                k = len(js)
                # forced pairs to fit 128 slots: pair same-node duplicates
                entries = []        # (row_id, node)
                if k > P:
                    need_pairs = k - P
                    bynode = {}
                    for u, j in zip(us.tolist(), js.tolist()):
                        bynode.setdefault(j, []).append(u)
                    items = sorted(bynode.items(),
                                   key=lambda kv: -len(kv[1]))
                    for j, ulist in items:
                        while need_pairs > 0 and len(ulist) >= 2:
                            u1, u2 = ulist.pop(), ulist.pop()
                            pair_rows.append((u1, u2))
                            entries.append((nrows + len(pair_rows) - 1, j))
                            need_pairs -= 1
                    assert need_pairs == 0, "pairing failed"
                    for j, ulist in items:
                        for u in ulist:
                            entries.append((int(u), j))
                else:
                    entries = list(zip(us.tolist(), js.tolist()))
                assert len(entries) <= P
                for pos, (row_id, j) in enumerate(entries):
                    slots_idx[r * P + pos] = row_id
                    sel[pos, r * 128 + j] = 1.0
            sel_all[t] = sel
            gsplit = tile_gsplit(t)
            idx_all[t] = np.tile(
                _wrap_idx(slots_idx.astype(np.int16), gsplit), (8, 1))
        npair = len(pair_rows)
        assert nrows + npair <= UMAX, (nrows, npair)
        emb_c = np.zeros((UMAX, 2 * D), dtype=fp8)
        emb_c[:U] = rows[0]
        if npair:
            pr = np.array(pair_rows, dtype=np.int64)
            emb_c[U:U + npair, :D] = eh[pr[:, 0]]
            emb_c[U:U + npair, D:] = eh[pr[:, 1]]
        in_maps.append({
            "emb": emb_c,
            "wsb": W_sb,
            "w8h": W8h,
            "w8l": W8l,
            "idx": np.ascontiguousarray(idx_all.reshape(NTILES * 128, IDXW)),
            "sel": np.ascontiguousarray(sel_all.reshape(NTILES * 128, NSLOT)),
        })
        perms.append(np.array(perm, dtype=np.int64))

    return in_maps, perms


# ---------------------------------------------------------------------------
# Numpy emulation (precision-faithful) for validation
# ---------------------------------------------------------------------------

def emulate_core(in_map):
    emb = in_map["emb"]                                        # [UMAX, 2D] fp8
    wsb = in_map["wsb"].reshape(128, RB, 2, D)                 # [p, j, c, o]
    w8h = in_map["w8h"].reshape(128, R8, 2, D).astype(np.float32)
    w8l = in_map["w8l"].reshape(128, R8, 2, D).astype(np.float32)
    idx = in_map["idx"].reshape(NTILES, 128, IDXW)
    sel = in_map["sel"].reshape(NTILES, 128, NSLOT)
    xfull = emb.astype(np.float32)
    xsum = xfull[:, :D] + xfull[:, D:]                         # plane0 + plane1
    out = np.zeros((NPC, D), dtype=np.float32)
    for t in range(NTILES):
        gsplit = tile_gsplit(t)
        parts, col = [], 0
        for a, b in zip(gsplit, gsplit[1:]):
            w = (b - a) * P // 16
            parts.append(idx[t, :16, col:col + w].T.reshape((b - a) * P))
            col += w
        slots_idx = np.concatenate(parts)                      # unwrap
        X = xsum[slots_idx]                                    # [NSLOT, D]
        out_acc = np.zeros((128, D), dtype=np.float32)
        for r in range(R1):
            selr = sel[t][:, r * 128:(r + 1) * 128].astype(np.float32)
            aggT = X[r * P:(r + 1) * P].T @ selr               # [D, 128] f32
            if r in CIDX:
                j = CIDX[r]
                hi = aggT.astype(fp8).astype(np.float32)
                for cc in range(2):
                    hc = hi[cc * 128:(cc + 1) * 128]
                    out_acc += hc.T @ (w8h[:, j, cc, :] + w8l[:, j, cc, :])
            else:
                j = BIDX[r]
                aggTb = aggT.astype(bf16).astype(np.float32)   # evac to bf16
                for cc in range(2):
                    out_acc += aggTb[cc * 128:(cc + 1) * 128].T @ \
                        wsb[:, j, cc, :].astype(np.float32)
        out[t * TN:(t + 1) * TN] = np.maximum(out_acc, 0.0).astype(
            bf16).astype(np.float32)
    return out


def emulate(emb_table, weights, neighbors, relations):
    in_maps, perms = prep(emb_table, weights, neighbors, relations)
    full = np.zeros((N, D), dtype=np.float32)
    for c in range(NCORES):
        full[perms[c]] = emulate_core(in_maps[c])
    return full


# ---------------------------------------------------------------------------
# Bass program
# ---------------------------------------------------------------------------

def build_program():
    import concourse.bacc as bacc
    import concourse.tile as tile
    import concourse.mybir as mybir

    nc = bacc.Bacc(
        "TRN2", target_bir_lowering=False, debug=False,
        num_devices=NCORES,
    )
    BF = mybir.dt.bfloat16
    F32 = mybir.dt.float32
    I16 = mybir.dt.int16
    F8 = mybir.dt.float8e4
    F8E5 = mybir.dt.float8e5
    DR = mybir.MatmulPerfMode.DoubleRow

    emb = nc.dram_tensor("emb", [UMAX, 2 * D], F8, kind="ExternalInput").ap()
    wsb = nc.dram_tensor("wsb", [128, RB * 2 * D], BF, kind="ExternalInput").ap()
    w8h = nc.dram_tensor("w8h", [128, R8 * 2 * D], F8, kind="ExternalInput").ap()
    w8l = nc.dram_tensor("w8l", [128, R8 * 2 * D], F8E5, kind="ExternalInput").ap()
    idx = nc.dram_tensor("idx", [NTILES * 128, IDXW], I16, kind="ExternalInput").ap()
    sel = nc.dram_tensor("sel", [NTILES * 128, NSLOT], F8,
                         kind="ExternalInput").ap()
    out = nc.dram_tensor("out", [NPC, D], BF, kind="ExternalOutput").ap()

    Relu = mybir.ActivationFunctionType.Relu

    with tile.TileContext(nc) as tc:
        with (
            tc.tile_pool(name="wpool", bufs=1) as wpool,
            tc.tile_pool(name="xpool0", bufs=1) as xpool0,
            tc.tile_pool(name="xpool", bufs=PF + 1) as xpool,
            tc.tile_pool(name="selpool0", bufs=1) as selpool0,
            tc.tile_pool(name="selpool", bufs=PF + 1) as selpool,
            tc.tile_pool(name="idxpool", bufs=PF + 1) as idxpool,
            tc.tile_pool(name="ypool", bufs=LAG + 3) as ypool,
            tc.tile_pool(name="ypool8", bufs=LAG + 3) as ypool8,
            tc.tile_pool(name="opool", bufs=2) as opool,
            tc.tile_pool(name="wupool", bufs=1) as wupool,
            tc.tile_pool(name="psy", bufs=SKEW + 2, space="PSUM") as psy,
            tc.tile_pool(name="pso", bufs=2, space="PSUM") as pso,
        ):
            wt = wpool.tile([128, RB * 2 * D], BF)
            w8ht = wpool.tile([128, R8 * 2 * D], F8)
            w8lt = wpool.tile([128, R8 * 2 * D], F8E5)

            def load_w(j0, j1):
                # packed bf16 weights (BF_RELS order)
                a, b = j0 * 2 * D, j1 * 2 * D
                nc.sync.dma_start(out=wt[:, a:b], in_=wsb[:, a:b])

            def load_w8(j0, j1, which):
                a, b = j0 * 2 * D, j1 * 2 * D
                if which == 0:
                    nc.sync.dma_start(out=w8ht[:, a:b], in_=w8h[:, a:b])
                else:
                    nc.sync.dma_start(out=w8lt[:, a:b], in_=w8l[:, a:b])

            # PE warmup: ramp the clock while the first gather lands.
            wu = wupool.tile([128, 128], BF)
            nc.vector.memset(wu[:], 0.0)
            wups = pso.tile([128, D], F32, name="outp")
            for _ in range(WARMUP):
                nc.tensor.matmul(out=wups[:, :128], lhsT=wu[:], rhs=wu[:],
                                 start=True, stop=True)

            # per-tile state: xs[t] = list of (g_lo, g_hi, tile);
            # sels[t] = list of (r_lo, r_hi, tile)
            xs, sels = {}, {}

            def pre_idx(t):
                idx_t = idxpool.tile([128, IDXW], I16, name="idx_t")
                nc.sync.dma_start(
                    out=idx_t[:], in_=idx[t * 128:(t + 1) * 128, :])
                return idx_t

            def gather_seg(t, idx_t, gi, a, b):
                n = (b - a) * P
                name = f"xt{gi}" if t == 0 else f"x{gi}"
                pool = xpool0 if t == 0 else xpool
                xt = pool.tile([128, b - a, 2 * D], F8, name=name)
                col = a * P // 16
                nc.gpsimd.dma_gather(
                    out_ap=xt[:],
                    in_ap=emb,
                    idxs_ap=idx_t[:, col:col + n // 16],
                    num_idxs=n,
                    num_idxs_reg=n,
                    elem_size=2 * D,
                    transpose=False,
                    single_packet=False,
                )
                xs.setdefault(t, []).append((a, b, xt))

            def load_sel(t, r0, r1, name, pool):
                st = pool.tile([128, (r1 - r0) * 128], F8, name=name)
                nc.sync.dma_start(
                    out=st[:],
                    in_=sel[t * 128:(t + 1) * 128, r0 * 128:r1 * 128])
                sels.setdefault(t, []).append((r0, r1, st))

            def prefetch(t):
                if t >= NTILES:
                    return
                idx_t = pre_idx(t)
                gs = tile_gsplit(t)
                for gi, (a, b) in enumerate(zip(gs, gs[1:])):
                    gather_seg(t, idx_t, gi, a, b)
                load_sel(t, 0, R1, "sel_t", selpool)

            def lookup(lst, r):
                for lo, hi, tl in lst:
                    if lo <= r < hi:
                        return lo, tl
                raise KeyError(r)

            # Startup orchestration: tile 0 finely segmented; W interleaved in
            # small chunks so the serial DMA engine feeds aggs, stage-B, and
            # the next tiles' gathers roughly in demand order.
            idx0 = pre_idx(0)

            # W loads in pair-demand order, popped between startup DMAs
            def pair_rels(k):
                return [r for r in (2 * k, 2 * k + 1) if r < R1]

            wload_ops = []
            for k in range(NPAIR):
                rs = pair_rels(k)
                if k in CONV_PAIRS:
                    j = CIDX[rs[0]]
                    wload_ops.append((load_w8, (j, j + len(rs), 0)))
                    wload_ops.append((load_w8, (j, j + len(rs), 1)))
                else:
                    j = BIDX[rs[0]]
                    wload_ops.append((load_w, (j, j + len(rs))))

            def pop_w(n):
                for _ in range(n):
                    if wload_ops:
                        f, args = wload_ops.pop(0)
                        f(*args)

            gather_seg(0, idx0, 0, *GSPLIT0[0:2])
            load_sel(0, 0, 6, "sel_a", selpool0)
            gather_seg(0, idx0, 1, *GSPLIT0[1:3])
            load_sel(0, 6, R1, "sel_b", selpool0)
            gather_seg(0, idx0, 2, *GSPLIT0[2:4])
            pop_w(3)
            pop_w(3)
            gather_seg(0, idx0, 3, *GSPLIT0[3:5])
            pop_w(3)
            gather_seg(0, idx0, 4, *GSPLIT0[4:6])
            pop_w(3)
            prefetch(1)
            pop_w(4)
            prefetch(2)
            pop_w(4)
            prefetch(3)
            pop_w(len(wload_ops))

            # Unified pair stream across all tiles: agg(g) runs SKEW pairs
            # ahead of stage-B(g); tile boundaries vanish (the next tile's
            # aggs fill the previous tile's stage-B drain window).
            NPT = NTILES * NPAIR
            cur, outps, ys = {}, {}, {}
            nextq = [0]

            def emit_stage_b():
                q = nextq[0]
                nextq[0] += 1
                qt, qk = q // NPAIR, q % NPAIR
                if qk == 0:
                    outps[qt] = pso.tile([128, D], F32, name="outp")
                outp = outps[qt]
                ysq = ys.pop(q)
                if qk in CONV_PAIRS:
                    ysbh = ysq
                    for u, r in enumerate(pair_rels(qk)):
                        j = CIDX[r]
                        wh = w8ht[:, j * 2 * D:(j + 1) * 2 * D].rearrange(
                            "p (c o) -> p c o", c=2)
                        wl = w8lt[:, j * 2 * D:(j + 1) * 2 * D].rearrange(
                            "p (c o) -> p c o", c=2)
                        for rhs in (wh, wl):
                            nc.tensor.matmul(
                                out=outp[:],
                                lhsT=ysbh[:, u, :, :],
                                rhs=rhs,
                                start=False, stop=False,
                                perf_mode=DR,
                            )
                else:
                    for u, r in enumerate(pair_rels(qk)):
                        for cc in range(2):
                            b = BIDX[r] * 2 + cc
                            nc.tensor.matmul(
                                out=outp[:],
                                lhsT=ysq[:, u, cc, :],
                                rhs=wt[:, b * D:(b + 1) * D],
                                start=(r == 0 and cc == 0),
                                stop=(r == R1 - 1 and cc == 1),
                            )
                if qk == NPAIR - 1:
                    outp = outps.pop(qt)
                    osb = opool.tile([128, D], BF)
                    nc.scalar.activation(out=osb[:], in_=outp[:], func=Relu)
                    nc.sync.dma_start(
                        out=out[qt * TN:(qt + 1) * TN, :], in_=osb[:])

            for g in range(NPT):
                t, k = g // NPAIR, g % NPAIR
                if k == 0:
                    cur[t] = (xs.pop(t), sels.pop(t))
                xlist, slist = cur[t]
                rs = pair_rels(k)
                yp = psy.tile([128, 2, 2, 128], F32, name="yp")
                for u, r in enumerate(rs):
                    slo, st = lookup(slist, r)
                    rhs = st[:, (r - slo) * 128:(r - slo + 1) * 128] \
                        .rearrange("p (u n) -> p u n", u=1) \
                        .broadcast_to([128, 2, 128])
                    glo, xt = lookup(xlist, r)
                    xr = xt[:, r - glo, :].rearrange(
                        "p (u d) -> p u d", u=2)
                    for cc in range(2):
                        nc.tensor.matmul(
                            out=yp[:, u, cc, :],
                            lhsT=xr[:, :, cc * 128:(cc + 1) * 128],
                            rhs=rhs,
                            start=True, stop=True,
                            perf_mode=DR,
                        )
                nu = len(rs)
                if k in CONV_PAIRS:
                    ysbh = ypool8.tile([128, 2, 2, 128], F8, name="ysbh")
                    if g % 2 == 0:
                        nc.vector.tensor_copy(out=ysbh[:, :nu], in_=yp[:, :nu])
                    else:
                        nc.scalar.copy(out=ysbh[:, :nu], in_=yp[:, :nu])
                    ys[g] = ysbh
                else:
                    ysb = ypool.tile([128, 2, 2, 128], BF, name="ysb")
                    if g % 2 == 0:
                        nc.vector.tensor_copy(out=ysb[:, :nu], in_=yp[:, :nu])
                    else:
                        nc.scalar.copy(out=ysb[:, :nu], in_=yp[:, :nu])
                    ys[g] = ysb
                if k == NPAIR - 1:
                    prefetch(t + PF + 1)
                lag = LAG if g < NPT - 2 * NPAIR else LAG_END
                while nextq[0] <= g - lag:
                    emit_stage_b()
            while nextq[0] < NPT:
                emit_stage_b()

    nc.compile()
    return nc


_NC_CACHE = []


def _get_program():
    if not _NC_CACHE:
        _NC_CACHE.append(build_program())
    return _NC_CACHE[0]


# ---------------------------------------------------------------------------
# Entry point
# ---------------------------------------------------------------------------

def kernel(emb_table, weights, neighbors, relations):
    from concourse import bass_utils

    in_maps, perms = prep(emb_table, weights, neighbors, relations)
    nc = _get_program()
    res = bass_utils.run_bass_kernel_spmd(
        nc, in_maps, core_ids=list(range(NCORES)),
    )
    full = np.zeros((N, D), dtype=np.float32)
    for c in range(NCORES):
        full[perms[c]] = np.asarray(res.results[c]["out"]).astype(np.float32)
    return full
